# revision 10
# baseline (speedup 1.0000x reference)
"""3x3 valid conv (single channel) on 8 TRN2 NeuronCores, fp16 I/O.

Strategy: memory-regime kernel, so halve HBM traffic by shipping X and Y as
fp16 (host converts; rel err ~8e-4, well under the 2e-2 gate). Per core the
conv is 3 banded matmuls per output tile accumulating in PSUM:
    out[m, c] = sum_dj (B_dj.T @ X_tile[:, c+dj])[m]
with B_dj[k, m] = W[k-m, dj] a [K, M] banded stationary built on host. fp16
matmuls run at 1 cyc/row directly on the DMA-loaded tiles (no f32r cast, so
the vector engine is free to share PSUM-drain duty with scalar).

Row split: 8 cores x 4 strips x 126 output rows = 4032 rows; the global
62-row tail strip (input rows 4032..4095) is column-split 8 ways and folded
2-up into 128 partitions (partitions 0..63 = cols [c0, c0+514), 64..127 =
cols [c0+512, c0+1026)) so one 512-wide matmul group covers 1024 output
columns of it. Tensor: ~100k cycles/core; DMA: ~16.6 MB/core.

Latency hiding: consts ride the idle SWDGE (gpsimd) ring so the SP ring
streams X from t=0 in 513 KB chunks; PE runs throwaway matmuls on a memset
scratch tile during the first chunk's flight so the HAM clock-gate is warm
(2.4 GHz) when real work arrives; a dummy activation preloads the ACT
table; the tail strip sits mid-stream so its store isn't on the critical
path. PSUM drains alternate scalar/vector so neither engine paces.
"""

import sys

sys.path.insert(0, "/opt/trn_rl_repo")

import numpy as np
from concourse import bass, mybir
from concourse.bass_utils import run_bass_kernel_spmd
from concourse.tile import TileContext

F16 = mybir.dt.float16
F32 = mybir.dt.float32

H, WIDTH = 4096, 8192
KH, KW = 3, 3
OH, OW = H - KH + 1, WIDTH - KW + 1       # 4094, 8190
N_CORES = 8
N_STRIPS = 4                              # full strips per core
SRPC = N_STRIPS * 126                     # 504 strip-output rows per core
IN_ROWS = SRPC + KH - 1                   # 506 input rows per core
TAIL_R0 = N_CORES * SRPC                  # 4032: first tail output row
TAIL_ROWS = OH - TAIL_R0                  # 62 tail output rows
TAIL_COLS = 1024                          # tail output cols per core (folded 2x512)
N_GROUPS = 8                              # 1024-col PSUM groups per strip
# strip-0 load chunk boundaries: small first chunk so the real matmul stream
# starts ASAP; later strips load in halves (fewer DMAs -> less DMAHW-lane
# serialization against the store DMAs, which share the 8 lanes)
S0_CHUNKS = [0, 514, 1026, 3076, 5126, 7176, WIDTH]
N_WARM = 10                               # HAM warmup matmuls (N=512 on scratch)


def _split_multi_waits(nc, max_waits=1):
    # This container's walrus rejects >1 sync-wait command per instruction
    # (CoreV3 setupSyncWait). Tile attaches one wait per producing logical
    # processor to a single instruction; hoist the excess onto same-engine
    # Drain carriers inserted immediately before it.
    for fn in nc.m.functions:
        for bb in fn.blocks:
            out = []
            changed = False
            for inst in bb.instructions:
                si = inst.sync_info
                waits = list(si.on_wait) if si and si.on_wait else []
                if len(waits) > max_waits:
                    rest = waits[max_waits:]
                    for j in range(0, len(rest), max_waits):
                        carrier = mybir.InstDrain(
                            name=nc.get_next_instruction_name(), ins=[], outs=[]
                        )
                        carrier.engine = inst.engine
                        carrier.sync_info = mybir.SyncInfo(
                            on_wait=rest[j : j + max_waits], on_update=[]
                        )
                        out.append(carrier)
                    si.on_wait = waits[:max_waits]
                    changed = True
                out.append(inst)
            if changed:
                bb.instructions = out


def _build(split_waits=True):
    nc = bass.Bass()
    xm = nc.declare_dram_parameter("xm", [IN_ROWS, WIDTH], F16, isOutput=False)
    xt = nc.declare_dram_parameter("xt", [128, 514], F16, isOutput=False)
    bands = nc.declare_dram_parameter("bands", [128, 3 * 128], F16, isOutput=False)
    bandt = nc.declare_dram_parameter("bandt", [128, 3 * 128], F16, isOutput=False)
    bias = nc.declare_dram_parameter("bias", [128, 1], F32, isOutput=False)
    ym = nc.declare_dram_parameter("ym", [SRPC, OW], F16, isOutput=True)
    yt = nc.declare_dram_parameter("yt", [TAIL_ROWS, TAIL_COLS], F16, isOutput=True)

    ident = mybir.ActivationFunctionType.Identity

    with TileContext(nc) as tc:
        with (
            tc.tile_pool(name="const", bufs=1) as cpool,
            tc.tile_pool(name="xin", bufs=4) as xpool,
            tc.tile_pool(name="stage", bufs=3) as spool,
            tc.tile_pool(name="psum", bufs=4, space="PSUM") as ppool,
        ):
            # consts ride the otherwise-idle SWDGE ring: they land in ~2-3us
            # while the SP ring streams X chunks uncontended
            band_t = cpool.tile([128, 3 * 128], F16)
            nc.gpsimd.dma_start(out=band_t[:], in_=bands[:])
            bandt_t = cpool.tile([128, 3 * 128], F16)
            nc.gpsimd.dma_start(out=bandt_t[:], in_=bandt[:])
            bias_t = cpool.tile([128, 1], F32)
            nc.gpsimd.dma_start(out=bias_t[:], in_=bias[:])
            xt_t = cpool.tile([128, 514], F16)
            nc.gpsimd.dma_start(out=xt_t[:], in_=xt[:])

            scratch = cpool.tile([128, 514], F16)
            nc.vector.memset(scratch[:], 0.0)
            scratch2 = cpool.tile([128, 16], F32)

            strip_tiles = []
            for s in range(N_STRIPS):
                xr = xpool.tile([128, WIDTH], F16, tag="xt")
                bounds = S0_CHUNKS if s == 0 else [0, 4096, WIDTH]
                for c0, c1 in zip(bounds, bounds[1:]):
                    nc.sync.dma_start(
                        out=xr[:, c0:c1],
                        in_=xm[126 * s : 126 * s + 128, c0:c1],
                    )
                strip_tiles.append(xr)

            # HAM warmup: throwaway N=512 matmuls on the scratch tile keep
            # the PE busy while chunk 0 is in flight, so the clock gate is
            # at 8/8 (2.4 GHz) when the real stream begins (N=512 keeps the
            # LDWEIGHTS duty low enough that HAM sees sustained activity).
            # The ACT table preload rides the same scratch.
            warm_ps = ppool.tile([128, 1024], F32, tag="ps")
            for i in range(N_WARM):
                nc.tensor.matmul(
                    warm_ps[:126, 0:512],
                    scratch[:, 0:126],
                    scratch[:, 0:512],
                    start=True,
                    stop=True,
                )
            nc.scalar.activation(
                scratch2[:, 0:16], scratch[:, 0:16], ident, bias=0.0, scale=1.0
            )

            def do_tail():
                ps = ppool.tile([128, 1024], F32, tag="ps")
                for dj in range(KW):
                    nc.tensor.matmul(
                        ps[:126, 0:512],
                        bandt_t[:, dj * 128 : dj * 128 + 126],
                        xt_t[:, dj : dj + 512],
                        start=(dj == 0),
                        stop=(dj == KW - 1),
                    )
                stage_t = spool.tile([128, 4096], F16, tag="stage")
                nc.scalar.activation(
                    stage_t[:126, 0:512],
                    ps[:126, 0:512],
                    ident,
                    bias=bias_t[:126, :],
                    scale=1.0,
                )
                nc.scalar.dma_start(out=yt[:, 0:512], in_=stage_t[0:TAIL_ROWS, 0:512])
                nc.scalar.dma_start(
                    out=yt[:, 512:1024], in_=stage_t[64 : 64 + TAIL_ROWS, 0:512]
                )

            for s in range(N_STRIPS):
                xr = strip_tiles[s]
                r0 = 126 * s
                for half in range(2):
                    stage = spool.tile([128, 4096], F16, tag="stage")
                    for gg in range(N_GROUPS // 2):
                        g = half * 4 + gg
                        ps = ppool.tile([128, 1024], F32, tag="ps")
                        for sub in range(2):
                            c0 = g * 1024 + sub * 512
                            n = min(512, OW - c0)
                            for dj in range(KW):
                                nc.tensor.matmul(
                                    ps[:126, sub * 512 : sub * 512 + n],
                                    band_t[:, dj * 128 : dj * 128 + 126],
                                    xr[:, c0 + dj : c0 + dj + n],
                                    start=(dj == 0),
                                    stop=(dj == KW - 1),
                                )
                        gw = min(1024, OW - g * 1024)
                        # alternate PSUM drains between scalar and vector so
                        # neither engine paces the pipeline
                        if g % 2 == 0:
                            nc.scalar.activation(
                                stage[:126, gg * 1024 : gg * 1024 + gw],
                                ps[:126, :gw],
                                ident,
                                bias=bias_t[:126, :],
                                scale=1.0,
                            )
                        else:
                            nc.vector.tensor_scalar_add(
                                stage[:126, gg * 1024 : gg * 1024 + gw],
                                ps[:126, :gw],
                                bias_t[:126, :],
                            )
                        if s == N_STRIPS - 1:
                            # last strip: store per drain so the final store
                            # is small and off the critical path
                            q0 = half * 4096 + gg * 1024
                            nc.scalar.dma_start(
                                out=ym[r0 : r0 + 126, q0 : q0 + gw],
                                in_=stage[0:126, gg * 1024 : gg * 1024 + gw],
                            )
                    if s < N_STRIPS - 1:
                        hw_ = min(4096, OW - half * 4096)
                        nc.scalar.dma_start(
                            out=ym[r0 : r0 + 126, half * 4096 : half * 4096 + hw_],
                            in_=stage[0:126, :hw_],
                        )
                if s == 1:
                    # tail sits mid-stream: its inputs landed early on the
                    # SWDGE ring and its store stays off the critical path
                    do_tail()

    if split_waits:
        _split_multi_waits(nc)
    return nc


_NC_CACHE = None


def _get_nc():
    global _NC_CACHE
    if _NC_CACHE is None:
        _NC_CACHE = _build()
    return _NC_CACHE


def _make_host_inputs(X, W, b):
    Xh = np.asarray(X, dtype=np.float32).astype(np.float16)
    W = np.asarray(W, dtype=np.float32)
    b = np.asarray(b, dtype=np.float32)

    # main band: B[k, dj*128 + m] = W[k-m, dj] for 0 <= k-m < 3, m < 126
    bands = np.zeros((128, 3 * 128), dtype=np.float32)
    for dj in range(KW):
        for dk in range(KH):
            mm = np.arange(126)
            bands[mm + dk, dj * 128 + mm] = W[dk, dj]
    # tail band: same rule restricted to the two folded blocks
    # (k 0..63 -> m 0..61, k 64..127 -> m 64..125)
    bandt = np.zeros((128, 3 * 128), dtype=np.float32)
    for dj in range(KW):
        for dk in range(KH):
            mm = np.arange(TAIL_ROWS)
            bandt[mm + dk, dj * 128 + mm] = W[dk, dj]
            bandt[64 + mm + dk, dj * 128 + 64 + mm] = W[dk, dj]
    bands = bands.astype(np.float16)
    bandt = bandt.astype(np.float16)
    bias = np.full((128, 1), float(b[0]), dtype=np.float32)

    in_maps = []
    for i in range(N_CORES):
        r0 = i * SRPC
        shard = np.ascontiguousarray(Xh[r0 : r0 + IN_ROWS])
        # tail fold: partitions 0..63 = rows 4032..4095 cols [c0, c0+514),
        # partitions 64..127 = same rows cols [c0+512, c0+1026), zero-padded
        # past the right edge of X (core 7); the padded outputs aren't stored.
        c0 = i * TAIL_COLS
        take = min(514 + 512, WIDTH - c0)
        tpad = np.zeros((64, 514 + 512), dtype=np.float16)
        tpad[:, :take] = Xh[TAIL_R0 : TAIL_R0 + 64, c0 : c0 + take]
        xt = np.empty((128, 514), dtype=np.float16)
        xt[0:64] = tpad[:, 0:514]
        xt[64:128] = tpad[:, 512:1026]
        in_maps.append(
            {"xm": shard, "xt": xt, "bands": bands, "bandt": bandt, "bias": bias}
        )
    return in_maps


def _assemble(results):
    out = np.empty((OH, OW), dtype=np.float32)
    for i in range(N_CORES):
        r0 = i * SRPC
        out[r0 : r0 + SRPC] = results[i]["ym"].astype(np.float32)
        c0 = i * TAIL_COLS
        w = min(TAIL_COLS, OW - c0)
        out[TAIL_R0:OH, c0 : c0 + w] = results[i]["yt"][:, :w].astype(np.float32)
    return out


def run(X, W, b, trace=False):
    nc = _get_nc()
    in_maps = _make_host_inputs(X, W, b)
    res = run_bass_kernel_spmd(nc, in_maps, list(range(N_CORES)), trace=trace)
    return _assemble(res.results), res


def kernel(X, W, b):
    out, _ = run(X, W, b)
    return out
